# revision 9
# baseline (speedup 1.0000x reference)
"""BottleneckAttention3D kernel for 8 Trainium2 NeuronCores.

Reference computation (per batch b):
    h = GroupNorm(x)                      # [C, N], C=128, N=4096, 8 groups
    q = wq @ h + bq ; k = wk @ h + bk ; v = wv @ h + bv
    attn = softmax(q.T k / sqrt(C))       # [N, N]
    out = v attn.T ; y = x + wp @ out + bp

Sharding: 8 cores = 2 batches x 4 query blocks of NQ=1024 tokens. Each core
recomputes stats + K/V for its whole batch, Q only for its query block, and
runs a flash-attention-style loop over 32 key blocks (scores never leave
PSUM/SBUF).

Key tricks:
  * GroupNorm affine h = s*x + t is folded on-device into the QKV weights
    (W' = W diag(s), b' = W t + b) so x feeds the matmuls directly.
  * rstd = exp(-0.5 ln(var+eps)) keeps all ACT work inside one table set
    (natural_log_exp_and_others), warmed by a dummy op at t=0.
  * No softmax max-subtraction (scores are O(6)); denominator = sum(exp),
    accumulated exactly in PSUM by ones-vector matmuls; 1/d = exp(-ln d).
  * The v bias drops out: sum_m attn = 1 makes it an additive constant,
    folded into the projection bias (wp @ bv' + bp).
  * Scores / attention*V / denominator matmuls run in bf16 (measured end
    to end max rel err ~3e-4); q/k projections stay on the f32r path.
"""

import sys

sys.path.insert(0, "/opt/trn_rl_repo")

import numpy as np

B = 2
C = 128
N = 4096  # 16*16*16 tokens
NQ = N // 4  # query block per core (1024)
GROUPS = 8
EPS = 1e-5
XCH = 1024  # x is DMAed in 4 chunks
NX = N // XCH  # 4
MB = N // 128  # 32 key blocks

_CACHE = {}


def _build():
    import concourse.bacc as bacc
    import concourse.mybir as mybir
    import concourse.tile as tile

    F32 = mybir.dt.float32
    F32R = mybir.dt.float32r
    BF16 = mybir.dt.bfloat16
    Exp = mybir.ActivationFunctionType.Exp
    Ln = mybir.ActivationFunctionType.Ln
    mult = mybir.AluOpType.mult
    add = mybir.AluOpType.add

    nc = bacc.Bacc("TRN2", target_bir_lowering=False, debug=False)

    # ---- DRAM I/O ----
    xb_d = nc.dram_tensor("xb", [C, N], F32R, kind="ExternalInput")
    xs_d = nc.dram_tensor("xs", [C, NQ], F32R, kind="ExternalInput")
    # blob_r: wqt_s | wkt | wvt | wpt | gm   (f32r, host-pretransposed)
    blr_d = nc.dram_tensor("blr", [C, 5 * C], F32R, kind="ExternalInput")
    # blob_f: bq_s | bk | bv | bp | gamma | beta   (f32 columns)
    blf_d = nc.dram_tensor("blf", [C, 6], F32, kind="ExternalInput")
    onb_d = nc.dram_tensor("onb", [C, 1], BF16, kind="ExternalInput")
    onr_d = nc.dram_tensor("onr", [1, C], F32R, kind="ExternalInput")
    y_d = nc.dram_tensor("y", [C, NQ], F32, kind="ExternalOutput")

    with tile.TileContext(nc) as tc:
        with (
            tc.tile_pool(name="cst", bufs=1) as cst,
            tc.tile_pool(name="xp", bufs=1) as xp,
            tc.tile_pool(name="ep", bufs=3) as ep,
            tc.tile_pool(name="psm", bufs=2, space="PSUM") as psm,
            tc.tile_pool(name="pso", bufs=1, space="PSUM") as pso,
        ):
            # dummy ACT op with no deps: triggers the one-and-only table
            # load (ln+exp set) while DMAs are still in flight
            DUM = cst.tile([1, 1], F32, tag="dum")
            nc.vector.memset(DUM, 1.0)
            DUM2 = cst.tile([1, 1], F32, tag="dum2")
            nc.scalar.activation(DUM2, DUM, Ln)

            # ---- input loads ----
            X = []
            for j in range(NX):
                xt = xp.tile([C, XCH], F32R, tag=f"x{j}", name=f"x{j}")
                nc.sync.dma_start(xt, xb_d[:, j * XCH : (j + 1) * XCH])
                X.append(xt)
            XS = cst.tile([C, NQ], F32R, tag="xs")
            nc.sync.dma_start(XS, xs_d[:, :])
            BLR = cst.tile([C, 5 * C], F32R, tag="blr")
            nc.sync.dma_start(BLR, blr_d[:, :])
            BLF = cst.tile([C, 6], F32, tag="blf")
            nc.gpsimd.dma_start(BLF, blf_d[:, :])
            ONB = cst.tile([C, 1], BF16, tag="onb")
            nc.gpsimd.dma_start(ONB, onb_d[:, :])
            ONR = cst.tile([1, C], F32R, tag="onr")
            nc.gpsimd.dma_start(ONR, onr_d[:, :])
            WQT = BLR[:, 0 * C : 1 * C]
            WKT = BLR[:, 1 * C : 2 * C]
            WVT = BLR[:, 2 * C : 3 * C]
            WPT = BLR[:, 3 * C : 4 * C]
            GM = BLR[:, 4 * C : 5 * C]
            BQ = BLF[:, 0:1]
            BK = BLF[:, 1:2]
            BV = BLF[:, 2:3]
            BP = BLF[:, 3:4]
            GAM = BLF[:, 4:5]
            BET = BLF[:, 5:6]

            # bf16 copies of x (feed the v-producing matmuls)
            XB = []
            for j in range(NX):
                xbt = xp.tile([C, XCH], BF16, tag=f"xb{j}", name=f"xb{j}")
                nc.gpsimd.tensor_copy(xbt, X[j].bitcast(F32))
                XB.append(xbt)

            # ---- group norm statistics ----
            ST = cst.tile([C, 2 * NX, 6], F32, tag="st")
            for j in range(NX):
                for h in range(2):
                    nc.vector.bn_stats(
                        out=ST[:, 2 * j + h, :],
                        in_=X[j][:, h * 512 : (h + 1) * 512].bitcast(F32),
                    )
            MV = cst.tile([C, 2], F32, tag="mv")
            nc.vector.bn_aggr(out=MV, in_=ST)

            # per-partition [mean, E[x^2]] -> group means via gm matmul
            S2 = cst.tile([C, 2], F32R, tag="s2")
            nc.vector.tensor_copy(S2[:, 0:1], MV[:, 0:1])
            # E[x^2] = mean*mean + var in one pass
            nc.vector.scalar_tensor_tensor(
                out=S2[:, 1:2],
                in0=MV[:, 0:1],
                scalar=MV[:, 0:1],
                in1=MV[:, 1:2],
                op0=mult,
                op1=add,
            )
            PG = psm.tile([C, 2], F32, tag="ps", name="pg")
            nc.tensor.matmul(PG, GM.bitcast(F32), S2.bitcast(F32), start=True, stop=True)
            GS = cst.tile([C, 2], F32, tag="gs")
            nc.vector.tensor_copy(GS, PG)
            # var_g = E_g[x^2] - mean_g^2  (negated: mean^2 - E, then *-1
            # folded into the Ln scale below via reciprocal... keep simple)
            NVG = cst.tile([C, 1], F32, tag="nvg")
            nc.vector.scalar_tensor_tensor(
                out=NVG,
                in0=GS[:, 0:1],
                scalar=GS[:, 0:1],
                in1=GS[:, 1:2],
                op0=mult,
                op1=mybir.AluOpType.subtract,
            )
            EPST = cst.tile([C, 1], F32, tag="epst")
            nc.vector.memset(EPST, float(EPS))
            # NVG = mean^2 - E[x^2] = -var; rstd = exp(-0.5 ln(-NVG + eps))
            LNV = cst.tile([C, 1], F32, tag="lnv")
            nc.scalar.activation(LNV, NVG, Ln, bias=EPST, scale=-1.0)
            RSTD = cst.tile([C, 1], F32, tag="rstd")
            nc.scalar.activation(RSTD, LNV, Exp, scale=-0.5)
            # s = rstd*gamma ; t = beta - mean_g*s
            SC = cst.tile([C, 1], F32, tag="sc")
            nc.vector.tensor_mul(SC, RSTD, GAM)
            NEGT = cst.tile([C, 1], F32R, tag="negt")
            nc.vector.scalar_tensor_tensor(
                out=NEGT,
                in0=GS[:, 0:1],
                scalar=SC,
                in1=BET,
                op0=mult,
                op1=mybir.AluOpType.subtract,
            )

            # ---- fold affine into weights: W' = W diag(s); b' = W t + b ----
            WQF = cst.tile([C, C], F32R, tag="wqf")
            WKF = cst.tile([C, C], F32R, tag="wkf")
            WVF = cst.tile([C, C], BF16, tag="wvf")
            nc.vector.tensor_scalar_mul(WQF, WQT, SC)
            nc.vector.tensor_scalar_mul(WKF, WKT, SC)
            nc.vector.tensor_scalar_mul(WVF, WVT, SC)
            PF = psm.tile([C, 4], F32, tag="ps", name="pf")
            nc.tensor.matmul(PF[:, 0:1], WQT.bitcast(F32), NEGT.bitcast(F32), start=True, stop=True)
            nc.tensor.matmul(PF[:, 1:2], WKT.bitcast(F32), NEGT.bitcast(F32), start=True, stop=True)
            nc.tensor.matmul(PF[:, 2:3], WVT.bitcast(F32), NEGT.bitcast(F32), start=True, stop=True)
            BQF = cst.tile([C, 1], F32, tag="bqf")
            BKF = cst.tile([C, 1], F32, tag="bkf")
            BVF = cst.tile([C, 1], F32R, tag="bvf")
            nc.vector.tensor_sub(BQF, BQ, PF[:, 0:1])
            nc.vector.tensor_sub(BKF, BK, PF[:, 1:2])
            nc.vector.tensor_sub(BVF, BV, PF[:, 2:3])
            # v-bias drops out of attention (rows of attn sum to 1); its
            # contribution is wp @ bv' added to the projection bias.
            PF2 = psm.tile([C, 1], F32, tag="ps", name="pf2")
            nc.tensor.matmul(PF2, WPT.bitcast(F32), BVF.bitcast(F32), start=True, stop=True)
            FB = cst.tile([C, 1], F32, tag="fb")
            nc.vector.tensor_add(FB, PF2, BP)
            XSB = cst.tile([C, NQ], F32, tag="xsb")
            nc.vector.tensor_scalar_add(XSB, XS.bitcast(F32), FB)

            # ---- K (bf16 [c,m] chunks), V (bf16 [m,c] blocks), Q ----
            K = []
            for j2 in range(2 * NX):
                pk = psm.tile([C, 512], F32, tag="ps", name=f"pk{j2}")
                nc.tensor.matmul(
                    pk, WKF, X[j2 // 2][:, (j2 % 2) * 512 : (j2 % 2 + 1) * 512],
                    start=True, stop=True,
                )
                kt = xp.tile([C, 512], BF16, tag=f"k{j2}", name=f"k{j2}")
                nc.vector.tensor_scalar_add(kt, pk, BKF)
                K.append(kt)
            V = []
            for g in range(2 * NX):
                pv = psm.tile([C, 4, 128], F32, tag="ps", name=f"pv{g}")
                for u in range(4):
                    m0 = (g % 2) * 512 + u * 128
                    nc.tensor.matmul(
                        pv[:, u, :],
                        XB[g // 2][:, m0 : m0 + 128],
                        WVF,
                        start=True,
                        stop=True,
                    )
                vt = xp.tile([C, 4, 128], BF16, tag=f"v{g}", name=f"v{g}")
                nc.vector.tensor_copy(vt, pv)
                V.append(vt)
            PQ = psm.tile([C, NQ], F32, tag="psq", name="pq")
            for h in range(2):
                sl = slice(h * 512, (h + 1) * 512)
                nc.tensor.matmul(PQ[:, sl], WQF, XS[:, sl], start=True, stop=True)
            QT = cst.tile([C, NQ], BF16, tag="qt")
            nc.vector.tensor_scalar_add(QT, PQ, BQF)

            # ---- main attention loop over 32 key blocks ----
            PO = pso.tile([C, NQ], F32, tag="po")
            PD = [
                psm.tile([1, 512], F32, tag="ps", name="pd0"),
                psm.tile([1, 512], F32, tag="ps", name="pd1"),
            ]
            for i in range(MB):
                g, u = i // 4, i % 4
                kblk = K[g][:, u * 128 : (u + 1) * 128]
                psS = psm.tile([C, NQ], F32, tag="psq", name=f"s{i}")
                for h in range(2):
                    sl = slice(h * 512, (h + 1) * 512)
                    nc.tensor.matmul(psS[:, sl], kblk, QT[:, sl], start=True, stop=True)
                E = ep.tile([C, NQ], BF16, tag="e", name=f"e{i}")
                nc.scalar.activation(E, psS, Exp)
                for h in range(2):
                    sl = slice(h * 512, (h + 1) * 512)
                    nc.tensor.matmul(
                        PO[:, sl], V[g][:, u, :], E[:, sl],
                        start=(i == 0), stop=(i == MB - 1),
                    )
                    nc.tensor.matmul(
                        PD[h], ONB, E[:, sl],
                        start=(i == 0), stop=(i == MB - 1),
                    )

            # ---- 1/denominator, normalize, project, residual ----
            LND = cst.tile([1, NQ], F32, tag="lnd")
            nc.scalar.activation(LND[:, 0:512], PD[0], Ln)
            nc.scalar.activation(LND[:, 512:1024], PD[1], Ln)
            RD = cst.tile([1, NQ], F32R, tag="rd")
            nc.scalar.activation(RD, LND, Exp, scale=-1.0)
            PB = psm.tile([C, NQ], F32, tag="psq", name="pb")
            for h in range(2):
                sl = slice(h * 512, (h + 1) * 512)
                nc.tensor.matmul(PB[:, sl], ONR, RD[:, sl], start=True, stop=True)
            RB = cst.tile([C, NQ], F32, tag="rb")
            nc.vector.tensor_copy(RB, PB)
            OUTN = cst.tile([C, NQ], F32R, tag="outn")
            nc.vector.tensor_mul(OUTN, PO, RB)
            PP = psm.tile([C, NQ], F32, tag="psq", name="pp")
            for h in range(2):
                sl = slice(h * 512, (h + 1) * 512)
                nc.tensor.matmul(PP[:, sl], WPT, OUTN[:, sl], start=True, stop=True)
            Y = cst.tile([C, NQ], F32, tag="y")
            nc.vector.tensor_add(Y, PP, XSB)
            nc.sync.dma_start(y_d[:, :], Y)

    nc.compile()
    return nc


def _get_nc():
    if "nc" not in _CACHE:
        _CACHE["nc"] = _build()
    return _CACHE["nc"]


def kernel(
    x,
    gamma,
    beta,
    wq,
    bq,
    wk,
    bk,
    wv,
    bv,
    wp,
    bp,
    _results_hook=None,
    _run_kwargs=None,
    **_unused,
):
    from concourse.bass_utils import run_bass_kernel_spmd

    f = np.float32
    x = np.ascontiguousarray(np.asarray(x, dtype=f))
    Bx, Cx, D, Hh, W = x.shape
    xr = x.reshape(Bx, Cx, D * Hh * W)

    gamma = np.asarray(gamma, f).reshape(C, 1)
    beta = np.asarray(beta, f).reshape(C, 1)
    wq = np.asarray(wq, f)
    wk = np.asarray(wk, f)
    wv = np.asarray(wv, f)
    wp = np.asarray(wp, f)
    bq = np.asarray(bq, f).reshape(C, 1)
    bk = np.asarray(bk, f).reshape(C, 1)
    bv = np.asarray(bv, f).reshape(C, 1)
    bp = np.asarray(bp, f).reshape(C, 1)

    scale = f(1.0) / np.sqrt(f(C))
    # blob_r: wqt_s | wkt | wvt | wpt | gm
    gsz = C // GROUPS
    gm = np.kron(np.eye(GROUPS, dtype=f), np.full((gsz, gsz), 1.0 / gsz, f))
    blr = np.concatenate(
        [
            np.ascontiguousarray(wq.T * scale),
            np.ascontiguousarray(wk.T),
            np.ascontiguousarray(wv.T),
            np.ascontiguousarray(wp.T),
            gm,
        ],
        axis=1,
    ).astype(f)
    blf = np.concatenate([bq * scale, bk, bv, bp, gamma, beta], axis=1).astype(f)

    import ml_dtypes

    shared = {
        "blr": blr,
        "blf": blf,
        "onb": np.ones((C, 1), ml_dtypes.bfloat16),
        "onr": np.ones((1, C), f),
    }
    in_maps = []
    for core in range(8):
        b, s = core // 4, core % 4
        in_maps.append(
            {
                "xb": np.ascontiguousarray(xr[b]),
                "xs": np.ascontiguousarray(xr[b][:, s * NQ : (s + 1) * NQ]),
                **shared,
            }
        )

    nc = _get_nc()
    res = run_bass_kernel_spmd(
        nc, in_maps, core_ids=list(range(8)), **(_run_kwargs or {})
    )
    if _results_hook is not None:
        _results_hook(res)

    out = np.empty((Bx, Cx, D * Hh * W), f)
    for core in range(8):
        b, s = core // 4, core % 4
        out[b][:, s * NQ : (s + 1) * NQ] = res.results[core]["y"]
    return out.reshape(Bx, Cx, D, Hh, W)
